# revision 13
# baseline (speedup 1.0000x reference)
"""Self-contained Trainium2 Bass kernel for a 3-layer dense transformer LM.

Model (fp32 reference): embed -> 3x[LN -> MHA(causal) -> +res -> LN -> FFN(gelu) -> +res]
-> LN -> logits.  B=2, S=1024, D=1024, H=16, HD=64, F=4096, V=32000.

Distribution over 8 NeuronCores (one TRN2 chip), Megatron-SP style with
AllToAll instead of reduce-scatter:
  - Residual stream is sequence-sharded: core c owns 256 tokens, kept
    TRANSPOSED in SBUF as rT [D=8x128 partitions-chunks, 256 tokens].
  - LN computed on local tokens (stats via ones-matmul over partition
    chunks), output all-gathered (bf16) so every core has xT [1024, 2048].
  - Attention tensor-parallel over heads: core c computes heads 2c,2c+1
    for ALL tokens; per-head causal softmax without max subtraction
    (scores are small); denominators come free via a ones-column in v.
  - AllToAll converts head-sharded attention output to token-sharded,
    then each core applies the FULL Wo for its own 256 tokens. Same
    pattern for FFN: W1 column-sharded, AllToAll, full W2 locally.
  - Logits: final LN -> AllGather -> each core computes a 4000-column
    vocab slice for all 2048 tokens; host concatenates.

Compute dtype bf16 (PE full rate), accumulation fp32 in PSUM.
"""

import numpy as np
import ml_dtypes

BF = ml_dtypes.bfloat16

B, S, D, H, L, F, V = 2, 1024, 1024, 16, 3, 4096, 32000
HD = D // H
T = B * S            # 2048 tokens
NC = 8               # cores
TLOC = T // NC       # 256 tokens per core
VS = V // NC         # 4000 vocab cols per core
EPS = 1e-5
DCH = D // 128       # 8 partition chunks of the hidden dim
FCH_LOC = F // NC // 128   # 4 chunks of the local FFN shard
VCH = 32             # vocab m-chunks per core
VMC = VS // VCH      # 125 vocab cols per m-chunk


def _build(n_layers, use_bout, ln_triv, debug=False):
    import concourse.bass as bass
    import concourse.mybir as mybir
    import concourse.tile as tile
    from concourse import bacc

    F32 = mybir.dt.float32
    BF16 = mybir.dt.bfloat16
    AF = mybir.ActivationFunctionType
    OP = mybir.AluOpType

    nc = bacc.Bacc("TRN2", target_bir_lowering=False, debug=False,
                   num_devices=NC)
    RG = [list(range(NC))]

    # ---------------- external parameters (per-core shards) ----------------
    ext = {}
    def inp(name, shape, dt=F32):
        ext[name] = nc.dram_tensor(name, shape, dt, kind="ExternalInput")
        return ext[name]

    embT = inp("embT", [128, DCH, TLOC])        # tok_emb[x_loc].T, f32
    posT = inp("posT", [128, DCH, TLOC])        # pos_emb slice .T, f32
    wq = inp("wq", [n_layers, 128, DCH, 128], BF16)   # pre-laid lhsT tiles
    wk = inp("wk", [n_layers, 128, DCH, 128], BF16)
    wv = inp("wv", [n_layers, 128, DCH, 128], BF16)
    wo = inp("wo", [n_layers, 128, DCH, D], BF16)
    w1 = inp("w1", [n_layers, 128, DCH, F // NC], BF16)
    w2 = inp("w2", [n_layers, DCH, 128, 32, 128], BF16)
    wout = inp("wout", [VCH, 128, DCH, VMC], BF16)
    bqkv = inp("bqkv", [128, 3, n_layers])      # [p, {q,k,v}, layer]
    boT = inp("boT", [128, DCH, n_layers])
    b1T = inp("b1T", [128, FCH_LOC, n_layers])
    b2T = inp("b2T", [128, DCH, n_layers])
    boutT = inp("boutT", [VMC, VCH])
    g1T = inp("g1T", [128, DCH, n_layers])
    be1T = inp("be1T", [128, DCH, n_layers])
    g2T = inp("g2T", [128, DCH, n_layers])
    be2T = inp("be2T", [128, DCH, n_layers])
    gfT = inp("gfT", [128, DCH, 1])
    befT = inp("befT", [128, DCH, 1])
    cmask = inp("cmask", [128, 2, TLOC], BF16)  # causal tiles f>=p / f>=p+128
    ident = inp("ident", [128, 128], BF16)

    outT = nc.dram_tensor("outT", [VS, T], F32, kind="ExternalOutput")

    dbg = {}
    def dbg_out(name, shape, dt=F32):
        if debug:
            dbg[name] = nc.dram_tensor(name, shape, dt, kind="ExternalOutput")
        return dbg.get(name)

    # ---------------- internal DRAM (collective bounce buffers) -----------
    ag_in, ag_out, a2a1_in, a2a1_out, a2a2_in, a2a2_out = [], [], [], [], [], []
    for i in range(n_layers):
        ag_in.append([nc.dram_tensor(f"ag{i}{j}_in", [128, DCH, TLOC], BF16)
                      for j in range(2)])
        ag_out.append([nc.dram_tensor(f"ag{i}{j}_out", [NC * 128, DCH, TLOC],
                                      BF16, addr_space="Shared")
                       for j in range(2)])
        a2a1_in.append(nc.dram_tensor(f"a2a1{i}_in", [NC, 128, TLOC], BF16))
        a2a1_out.append(nc.dram_tensor(f"a2a1{i}_out", [NC, 128, TLOC], BF16))
        a2a2_in.append(nc.dram_tensor(f"a2a2{i}_in", [NC, 512, TLOC], BF16))
        a2a2_out.append(nc.dram_tensor(f"a2a2{i}_out", [NC, 512, TLOC], BF16))
    agf_in = nc.dram_tensor("agf_in", [128, DCH, TLOC], BF16)
    agf_out = nc.dram_tensor("agf_out", [NC * 128, DCH, TLOC], BF16,
                             addr_space="Shared")

    with tile.TileContext(nc) as tc:
        with tc.tile_pool(name="const", bufs=1) as konst, \
             tc.tile_pool(name="resid", bufs=1) as resid, \
             tc.tile_pool(name="acts", bufs=1) as acts, \
             tc.tile_pool(name="wpool", bufs=2) as wpool, \
             tc.tile_pool(name="wop", bufs=3) as wop, \
             tc.tile_pool(name="lnp", bufs=1) as lnp, \
             tc.tile_pool(name="otp", bufs=2) as otp, \
             tc.tile_pool(name="att", bufs=2) as att, \
             tc.tile_pool(name="tmp", bufs=3) as tmp, \
             tc.tile_pool(name="expp", bufs=3) as expp, \
             tc.tile_pool(name="ps", bufs=2, space="PSUM") as ps, \
             tc.tile_pool(name="ps1", bufs=1, space="PSUM") as ps1:
            pass

            # ---- constants resident in SBUF ----
            msk = konst.tile([128, 2, TLOC], BF16)
            nc.scalar.dma_start(out=msk[:], in_=cmask[:, :, :])
            idn = konst.tile([128, 128], BF16)
            nc.scalar.dma_start(out=idn[:], in_=ident[:, :])
            ones_bf = konst.tile([128, 1], BF16)
            nc.vector.memset(ones_bf[:], 1.0)
            bqkv_sb = konst.tile([128, 3, n_layers], F32)
            nc.scalar.dma_start(out=bqkv_sb[:], in_=bqkv[:, :, :])
            bo_sb = konst.tile([128, DCH, n_layers], F32)
            nc.scalar.dma_start(out=bo_sb[:], in_=boT[:, :, :])
            b1_sb = konst.tile([128, FCH_LOC, n_layers], F32)
            nc.scalar.dma_start(out=b1_sb[:], in_=b1T[:, :, :])
            b2_sb = konst.tile([128, DCH, n_layers], F32)
            nc.scalar.dma_start(out=b2_sb[:], in_=b2T[:, :, :])
            lng = {}
            for nm, t_ in (("g1", g1T), ("be1", be1T), ("g2", g2T),
                           ("be2", be2T), ("gf", gfT), ("bef", befT)):
                nl = 1 if nm in ("gf", "bef") else n_layers
                lnt = konst.tile([128, DCH, nl], F32, tag=f"ln_{nm}", name=f"ln_{nm}")
                lng[nm] = lnt
                nc.scalar.dma_start(out=lng[nm][:], in_=t_[:, :, :])

            # ---- residual init: rT = embT + posT ----
            rT = resid.tile([128, DCH, TLOC], F32)
            pt = acts.tile([128, DCH, TLOC], F32, tag="scratch8k")
            nc.sync.dma_start(out=rT[:], in_=embT[:, :, :])
            nc.sync.dma_start(out=pt[:], in_=posT[:, :, :])
            nc.vector.tensor_add(rT[:], rT[:], pt[:])
            if debug:
                o = dbg_out("dbg_rT0", [128, DCH, TLOC])
                nc.sync.dma_start(out=o[:, :, :], in_=rT[:])

            # ---------------- helpers ----------------
            def layernorm(gname, bname, li, triv, out_dram):
                """LN over the partition-chunk dim of rT -> bf16 xln tile,
                DMA'd into out_dram ([128, DCH, TLOC])."""
                xb = lnp.tile([128, DCH, TLOC], BF16, tag="ln_xb")
                nc.vector.tensor_copy(xb[:], rT[:])
                sq = lnp.tile([128, DCH, TLOC], BF16, tag="ln_sq")
                nc.scalar.square(sq[:], rT[:])
                st = ps1.tile([1, 2 * TLOC], mybir.dt.float32, tag="pstat")
                for c in range(DCH):
                    nc.tensor.matmul(st[:, 0:TLOC], ones_bf[:], xb[:, c, :],
                                     start=(c == 0), stop=(c == DCH - 1))
                for c in range(DCH):
                    nc.tensor.matmul(st[:, TLOC:], ones_bf[:], sq[:, c, :],
                                     start=(c == 0), stop=(c == DCH - 1))
                m1 = tmp.tile([1, TLOC], mybir.dt.float32, tag="ln_m1")
                nc.vector.tensor_scalar_mul(m1[:], st[:, 0:TLOC], 1.0 / D)
                m2 = tmp.tile([1, TLOC], mybir.dt.float32, tag="ln_m2")
                nc.vector.tensor_scalar_mul(m2[:], st[:, TLOC:], 1.0 / D)
                t2 = tmp.tile([1, TLOC], mybir.dt.float32, tag="ln_t2")
                nc.vector.tensor_mul(t2[:], m1[:], m1[:])
                nc.vector.tensor_sub(m2[:], m2[:], t2[:])
                nc.vector.tensor_scalar_add(m2[:], m2[:], EPS)
                nc.scalar.activation(t2[:], m2[:], AF.Sqrt)
                rstd = m2
                nc.vector.reciprocal(rstd[:], t2[:])
                off = t2
                nc.vector.tensor_mul(off[:], m1[:], rstd[:])
                nc.vector.tensor_scalar_mul(off[:], off[:], -1.0)
                rstd_b = lnp.tile([128, TLOC], mybir.dt.float32, tag="ln_rb")
                nc.gpsimd.partition_broadcast(rstd_b[:], rstd[:], channels=128)
                off_b = lnp.tile([128, TLOC], mybir.dt.float32, tag="ln_ob")
                nc.gpsimd.partition_broadcast(off_b[:], off[:], channels=128)
                xo = lnp.tile([128, DCH, TLOC], BF16, tag="ln_out")
                for c in range(DCH):
                    t_ = lnp.tile([128, TLOC], mybir.dt.float32, tag="ln_t")
                    nc.vector.tensor_mul(t_[:], rT[:, c, :], rstd_b[:])
                    if triv:
                        nc.vector.tensor_add(xo[:, c, :], t_[:], off_b[:])
                    else:
                        nc.vector.tensor_add(t_[:], t_[:], off_b[:])
                        nc.vector.tensor_scalar(
                            xo[:, c, :], t_[:],
                            lng[gname][:, c, li:li + 1],
                            lng[bname][:, c, li:li + 1],
                            op0=OP.mult, op1=OP.add)
                nc.sync.dma_start(out=out_dram[:, :, :], in_=xo[:])
                return xo

            def load_xtf(src):
                """AllGather output -> SBUF [128, DCH, NC, TLOC] (bf16)."""
                xtf = acts.tile([128, DCH, NC, TLOC], BF16, tag="bigact")
                v = src.ap().rearrange("(r p) c t -> p c r t", p=128)
                nc.sync.dma_start(out=xtf[:], in_=v)
                return xtf

            # =================== layers ===================
            for li in range(n_layers):
                # ---- LN1 + AllGather ----
                layernorm("g1", "be1", li, ln_triv[2 * li], ag_in[li][0])
                nc.gpsimd.collective_compute(
                    "AllGather", OP.bypass, replica_groups=RG,
                    ins=[ag_in[li][0].ap().opt()],
                    outs=[ag_out[li][0].ap().opt()])
                xtf = load_xtf(ag_out[li][0])
                if debug and li == 0:
                    o = dbg_out("dbg_xtf", [128, DCH, NC, TLOC], BF16)
                    nc.sync.dma_start(out=o[:, :, :, :], in_=xtf[:])

                # ---- QKV projections ----
                wq_sb = wpool.tile([128, DCH, 128], BF16, tag="wq")
                wk_sb = wpool.tile([128, DCH, 128], BF16, tag="wk")
                wv_sb = wpool.tile([128, DCH, 128], BF16, tag="wv")
                for wsb, wext in ((wq_sb, wq), (wk_sb, wk), (wv_sb, wv)):
                    nc.scalar.dma_start(out=wsb[:], in_=wext[li, :, :, :])
                qT = acts.tile([128, T], BF16, tag="qT")
                kT = acts.tile([128, T], BF16, tag="kT")
                vT = acts.tile([128, T], BF16, tag="vT")
                for j, (wsb, dst) in enumerate(
                        ((wq_sb, qT), (wk_sb, kT), (wv_sb, vT))):
                    for np_ in range(2):      # pairs of 512-token slices
                        pps = [ps.tile([128, 512], mybir.dt.float32,
                                       tag="p512", name=f"pp{j}{np_}{k}")
                               for k in range(2)]
                        for d in range(DCH):
                            for k in range(2):
                                n = 2 * np_ + k
                                nc.tensor.matmul(
                                    pps[k][:],
                                    wsb[:, d, :],
                                    xtf[:, d, 2 * n:2 * n + 2, :],
                                    start=(d == 0), stop=(d == DCH - 1))
                        for k in range(2):
                            n = 2 * np_ + k
                            nc.vector.tensor_scalar(
                                dst[:, 512 * n:512 * (n + 1)], pps[k][:],
                                bqkv_sb[:, j, li:li + 1], None, op0=OP.add)
                # v: transpose vT -> v_aug [128, 16 tok-tiles, 130]
                v_aug = acts.tile([128, 16, 130], BF16, tag="vaug")
                nc.vector.memset(v_aug[:, :, 64:65], 1.0)
                nc.vector.memset(v_aug[:, :, 129:130], 1.0)
                for tt in range(16):
                    tp = ps1.tile([128, 128], BF16, tag="ptr")
                    nc.tensor.transpose(tp[:], vT[:, 128 * tt:128 * (tt + 1)],
                                        idn[:])
                    nc.vector.tensor_copy(v_aug[:, tt, 0:64], tp[:, 0:64])
                    nc.vector.tensor_copy(v_aug[:, tt, 65:129], tp[:, 64:128])
                if debug and li == 0:
                    for nm, t_ in (("dbg_qT", qT), ("dbg_kT", kT), ("dbg_vaug", v_aug)):
                        o = dbg_out(nm, list(t_.shape), BF16)
                        nc.sync.dma_start(out=o.ap().opt(), in_=t_[:])

                # ---- attention (2 local heads x 2 batches) ----
                hoT = acts.tile([64, 2, T], BF16, tag="hoT")  # 8KB
                for b in range(B):
                    for h in range(2):
                        hb = 64 * h
                        dnc = att.tile([1, 1024], mybir.dt.float32, tag="dnc")
                        for qs in range(4):    # q slices of 256
                            av = ps.tile([65, TLOC], mybir.dt.float32,
                                         tag="pav")
                            nkc = 2 * qs + 2
                            for kc in range(nkc):
                                sc = ps.tile([128, TLOC], mybir.dt.float32,
                                             tag="psc")
                                nc.tensor.matmul(
                                    sc[:],
                                    kT[hb:hb + 64,
                                       1024 * b + 128 * kc:
                                       1024 * b + 128 * (kc + 1)],
                                    qT[hb:hb + 64,
                                       1024 * b + 256 * qs:
                                       1024 * b + 256 * (qs + 1)],
                                    start=True, stop=True)
                                et_ = expp.tile([128, TLOC], BF16, tag="exp")
                                nc.scalar.activation(et_[:], sc[:], AF.Exp,
                                                     scale=float(HD) ** -0.5)
                                if kc >= 2 * qs:   # diagonal pair -> mask
                                    mi = 0 if kc == 2 * qs else 1
                                    nc.vector.tensor_mul(et_[:], et_[:],
                                                         msk[:, mi, :])
                                nc.tensor.matmul(
                                    av[:],
                                    v_aug[:, 8 * b + kc,
                                          65 * h:65 * (h + 1)],
                                    et_[:],
                                    start=(kc == 0), stop=(kc == nkc - 1))
                            nc.vector.tensor_copy(
                                dnc[:, 256 * qs:256 * (qs + 1)], av[64:65, :])
                            nc.vector.tensor_copy(
                                hoT[:, h,
                                    1024 * b + 256 * qs:
                                    1024 * b + 256 * (qs + 1)],
                                av[0:64, :])
                        nc.vector.reciprocal(dnc[:], dnc[:])
                        rcb = att.tile([64, 1024], mybir.dt.float32, tag="rcb")
                        nc.gpsimd.partition_broadcast(rcb[:], dnc[:],
                                                      channels=64)
                        nc.vector.tensor_mul(
                            hoT[:, h, 1024 * b:1024 * (b + 1)],
                            hoT[:, h, 1024 * b:1024 * (b + 1)], rcb[:])
                if debug and li == 0:
                    o = dbg_out("dbg_hoT", [64, 2, T], BF16)
                    nc.sync.dma_start(out=o[:, :, :], in_=hoT[:])

                # ---- AllToAll (head-sharded -> token-sharded) + Wo ----
                for h in range(2):
                    nc.sync.dma_start(
                        out=a2a1_in[li].ap().rearrange(
                            "r (h p) t -> p h r t", p=64)[:, h, :, :],
                        in_=hoT[:, h, :].rearrange("p (r t) -> p r t", r=NC))
                nc.gpsimd.collective_compute(
                    "AllToAll", OP.bypass, replica_groups=RG,
                    ins=[a2a1_in[li].ap().opt()],
                    outs=[a2a1_out[li].ap().opt()])
                ac = acts.tile([128, NC, TLOC], BF16, tag="hoac")
                nc.sync.dma_start(
                    out=ac[:],
                    in_=a2a1_out[li].ap().rearrange("r p t -> p r t"))
                for m in range(DCH):
                    wo_sb = wop.tile([128, DCH, 128], BF16, tag="wom")
                    nc.scalar.dma_start(out=wo_sb[:],
                                      in_=wo[li, :, :, 128 * m:128 * (m + 1)])
                    pp = ps.tile([128, TLOC], mybir.dt.float32, tag="psc")
                    for kc in range(DCH):
                        nc.tensor.matmul(pp[:],
                                         wo_sb[:, kc, :],
                                         ac[:, kc, :],
                                         start=(kc == 0), stop=(kc == DCH - 1))
                    nc.vector.scalar_tensor_tensor(
                        rT[:, m, :], pp[:], bo_sb[:, m, li:li + 1], rT[:, m, :],
                        op0=OP.add, op1=OP.add)
                if debug and li == 0:
                    o = dbg_out("dbg_rT1", [128, DCH, TLOC])
                    nc.sync.dma_start(out=o[:, :, :], in_=rT[:])

                # ---- LN2 + AllGather + W1 + gelu ----
                layernorm("g2", "be2", li, ln_triv[2 * li + 1], ag_in[li][1])
                nc.gpsimd.collective_compute(
                    "AllGather", OP.bypass, replica_groups=RG,
                    ins=[ag_in[li][1].ap().opt()],
                    outs=[ag_out[li][1].ap().opt()])
                xtf2 = load_xtf(ag_out[li][1])
                w1_sb = acts.tile([128, DCH, F // NC], BF16, tag="w1")
                nc.scalar.dma_start(out=w1_sb[:], in_=w1[li, :, :, :])
                h1T = acts.tile([128, FCH_LOC, T], BF16, tag="h1T")
                for m in range(FCH_LOC):
                    for np_ in range(2):
                        pps = [ps.tile([128, 512], mybir.dt.float32,
                                       tag="p512", name=f"pw1{m}{np_}{k}")
                               for k in range(2)]
                        for d in range(DCH):
                            for k in range(2):
                                n = 2 * np_ + k
                                nc.tensor.matmul(
                                    pps[k][:],
                                    w1_sb[:, d, 128 * m:128 * (m + 1)],
                                    xtf2[:, d, 2 * n:2 * n + 2, :],
                                    start=(d == 0), stop=(d == DCH - 1))
                        for k in range(2):
                            n = 2 * np_ + k
                            nc.scalar.activation(
                                h1T[:, m, 512 * n:512 * (n + 1)],
                                pps[k][:], AF.Gelu,
                                bias=b1_sb[:, m, li:li + 1])
                if debug and li == 0:
                    o = dbg_out("dbg_h1T", [128, FCH_LOC, T], BF16)
                    nc.sync.dma_start(out=o[:, :, :], in_=h1T[:])

                # ---- AllToAll + full W2 ----
                for m in range(FCH_LOC):
                    nc.sync.dma_start(
                        out=a2a2_in[li].ap().rearrange(
                            "r (m p) t -> p m r t", p=128)[:, m, :, :],
                        in_=h1T[:, m, :].rearrange("p (r t) -> p r t", r=NC))
                nc.gpsimd.collective_compute(
                    "AllToAll", OP.bypass, replica_groups=RG,
                    ins=[a2a2_in[li].ap().opt()],
                    outs=[a2a2_out[li].ap().opt()])
                h2 = acts.tile([128, 32, TLOC], BF16, tag="bigact")
                for m in range(FCH_LOC):
                    nc.sync.dma_start(
                        out=h2[:, m::FCH_LOC, :],
                        in_=a2a2_out[li].ap().rearrange(
                            "r (m p) t -> p m r t", p=128)[:, m, :, :])
                for m in range(DCH):
                    w2_sb = wop.tile([128, 32, 128], BF16, tag="w2")
                    nc.scalar.dma_start(out=w2_sb[:], in_=w2[li, m, :, :, :])
                    pp = ps.tile([128, TLOC], mybir.dt.float32, tag="psc")
                    for kc in range(32):
                        nc.tensor.matmul(pp[:], w2_sb[:, kc, :], h2[:, kc, :],
                                         start=(kc == 0), stop=(kc == 31))
                    nc.vector.scalar_tensor_tensor(
                        rT[:, m, :], pp[:], b2_sb[:, m, li:li + 1], rT[:, m, :],
                        op0=OP.add, op1=OP.add)
                if debug and li == 0:
                    o = dbg_out("dbg_rT2", [128, DCH, TLOC])
                    nc.sync.dma_start(out=o[:, :, :], in_=rT[:])

            # =================== final LN + logits ===================
            layernorm("gf", "bef", 0, ln_triv[2 * n_layers], agf_in)
            nc.gpsimd.collective_compute(
                "AllGather", OP.bypass, replica_groups=RG,
                ins=[agf_in.ap().opt()], outs=[agf_out.ap().opt()])
            xtff = load_xtf(agf_out)
            bout_sb = None
            if use_bout:
                bout_sb = konst.tile([VMC, VCH], mybir.dt.float32)
                nc.scalar.dma_start(out=bout_sb[:], in_=boutT[:, :])
            for m in range(VCH):
                wo_t = wop.tile([128, DCH, VMC], BF16, tag="wout")
                nc.scalar.dma_start(out=wo_t[:], in_=wout[m, :, :, :])
                ot = otp.tile([VMC, T], mybir.dt.float32, tag="ot")
                for np_ in range(2):
                    pps = [ps.tile([VMC, 512], mybir.dt.float32,
                                   tag="p512", name=f"plg{np_}{k}")
                           for k in range(2)]
                    for d in range(DCH):
                        for k in range(2):
                            n = 2 * np_ + k
                            nc.tensor.matmul(pps[k][:], wo_t[:, d, :],
                                             xtff[:, d, 2 * n:2 * n + 2, :],
                                             start=(d == 0),
                                             stop=(d == DCH - 1))
                    for k in range(2):
                        n = 2 * np_ + k
                        if use_bout:
                            nc.vector.tensor_scalar(
                                ot[:, 512 * n:512 * (n + 1)], pps[k][:],
                                bout_sb[:, m:m + 1], None, op0=OP.add)
                        else:
                            nc.scalar.copy(ot[:, 512 * n:512 * (n + 1)],
                                           pps[k][:])
                nc.sync.dma_start(out=outT[VMC * m:VMC * (m + 1), :],
                                  in_=ot[:])

    nc.compile()
    return nc, sorted(dbg.keys())


def _prep_inputs(inputs):
    """Shard + transform full inputs -> list of 8 per-core in_maps."""
    f32 = lambda a: np.ascontiguousarray(np.asarray(a), dtype=np.float32)
    bf16 = lambda a: np.ascontiguousarray(np.asarray(a, dtype=np.float32)).astype(BF)

    x = np.asarray(inputs["x"]).astype(np.int64).reshape(T)
    tok = f32(inputs["tok_emb"])
    pos = f32(inputs["pos_emb"])
    Wq, Wk, Wv, Wo = (f32(inputs[k]) for k in ("Wq", "Wk", "Wv", "Wo"))
    W1, W2, Wout = f32(inputs["W1"]), f32(inputs["W2"]), f32(inputs["Wout"])
    bq, bk, bv = f32(inputs["bq"]), f32(inputs["bk"]), f32(inputs["bv"])
    bo, b1, b2 = f32(inputs["bo"]), f32(inputs["b1"]), f32(inputs["b2"])
    bout = f32(inputs["bout"])
    ln1_g, ln1_b = f32(inputs["ln1_g"]), f32(inputs["ln1_b"])
    ln2_g, ln2_b = f32(inputs["ln2_g"]), f32(inputs["ln2_b"])
    lnf_g, lnf_b = f32(inputs["lnf_g"]), f32(inputs["lnf_b"])

    def chunkT(vec_1d):  # [D] -> [128, DCH] (p, chunk)
        return np.ascontiguousarray(vec_1d.reshape(DCH, 128).T)

    # causal mask tiles: keep if f >= p + 128*r
    p_i = np.arange(128)[:, None]
    f_i = np.arange(TLOC)[None, :]
    cm = np.stack([(f_i >= p_i), (f_i >= p_i + 128)], 1).astype(BF)  # [128,2,256]
    idn = np.eye(128, dtype=BF)

    ln_triv = []
    for g, b in ((ln1_g, ln1_b), (ln2_g, ln2_b)):
        for i in range(L):
            ln_triv.append(bool(np.all(g[i] == 1.0) and np.all(b[i] == 0.0)))
    # interleave per layer: [ln1_0, ln2_0, ln1_1, ln2_1, ...]
    ln_triv = [ln_triv[i] if s == 0 else ln_triv[L + i]
               for i in range(L) for s in range(2)] + \
              [bool(np.all(lnf_g == 1.0) and np.all(lnf_b == 0.0))]
    use_bout = bool(np.any(bout))

    in_maps = []
    for c in range(NC):
        xl = x[TLOC * c:TLOC * (c + 1)]
        embT = np.ascontiguousarray(tok[xl].T).reshape(DCH, 128, TLOC)
        embT = np.ascontiguousarray(embT.transpose(1, 0, 2))  # [128, DCH, 256]
        p0 = (c % 4) * TLOC
        posT = np.ascontiguousarray(pos[p0:p0 + TLOC].T).reshape(DCH, 128, TLOC)
        posT = np.ascontiguousarray(posT.transpose(1, 0, 2))
        hc = slice(128 * c, 128 * (c + 1))
        lay = lambda w: np.ascontiguousarray(
            w.reshape(L, DCH, 128, w.shape[-1]).transpose(0, 2, 1, 3))
        w2h = np.ascontiguousarray(
            W2.reshape(L, 32, 128, DCH, 128).transpose(0, 3, 2, 1, 4))
        wouth = np.ascontiguousarray(
            Wout[:, VS * c:VS * (c + 1)].reshape(DCH, 128, VCH, VMC)
            .transpose(2, 1, 0, 3))
        m = dict(
            embT=embT, posT=posT,
            wq=bf16(lay(Wq[:, :, hc])), wk=bf16(lay(Wk[:, :, hc])),
            wv=bf16(lay(Wv[:, :, hc])),
            wo=bf16(lay(Wo)), w1=bf16(lay(W1[:, :, 512 * c:512 * (c + 1)])),
            w2=bf16(w2h), wout=bf16(wouth),
            bqkv=np.ascontiguousarray(
                np.stack([bq[:, hc], bk[:, hc], bv[:, hc]], 1).T),  # [128,3,L]
            boT=np.ascontiguousarray(
                np.stack([chunkT(bo[i]) for i in range(L)], -1)),
            b1T=np.ascontiguousarray(np.stack(
                [np.ascontiguousarray(
                    b1[i, 512 * c:512 * (c + 1)].reshape(FCH_LOC, 128).T)
                 for i in range(L)], -1)),
            b2T=np.ascontiguousarray(
                np.stack([chunkT(b2[i]) for i in range(L)], -1)),
            boutT=np.ascontiguousarray(
                bout[VS * c:VS * (c + 1)].reshape(VCH, VMC).T),
            g1T=np.ascontiguousarray(
                np.stack([chunkT(ln1_g[i]) for i in range(L)], -1)),
            be1T=np.ascontiguousarray(
                np.stack([chunkT(ln1_b[i]) for i in range(L)], -1)),
            g2T=np.ascontiguousarray(
                np.stack([chunkT(ln2_g[i]) for i in range(L)], -1)),
            be2T=np.ascontiguousarray(
                np.stack([chunkT(ln2_b[i]) for i in range(L)], -1)),
            gfT=chunkT(lnf_g)[:, :, None].copy(),
            befT=chunkT(lnf_b)[:, :, None].copy(),
            cmask=cm, ident=idn,
        )
        in_maps.append(m)
    return in_maps, use_bout, ln_triv


_CACHE = {}


def _get_nc(n_layers, use_bout, ln_triv, debug=False):
    key = (n_layers, use_bout, tuple(ln_triv), debug)
    if key not in _CACHE:
        _CACHE[key] = _build(n_layers, use_bout, ln_triv, debug)
    return _CACHE[key]


def run(inputs, n_layers=L, debug=False, trace=False):
    from concourse import bass_utils
    in_maps, use_bout, ln_triv = _prep_inputs(inputs)
    nc, dbg_names = _get_nc(n_layers, use_bout, ln_triv, debug)
    res = bass_utils.run_bass_kernel_spmd(
        nc, in_maps, core_ids=list(range(NC)), trace=trace)
    return res, dbg_names


def kernel(**inputs):
    res, _ = run(inputs)
    outs = [res.results[c]["outT"] for c in range(NC)]          # [4000, 2048]
    logits = np.concatenate([o.T for o in outs], axis=1)        # [2048, 32000]
    return np.ascontiguousarray(logits.reshape(B, S, V))
